# revision 7
# baseline (speedup 1.0000x reference)
"""Bass/Trainium2 kernel for masked multi-head attention with coverage.

Problem shapes (hardcoded): B=4, LQ=LK=2048, D=512, H=8, DH=64.
Sharding: 8 cores = (batch b in 0..3) x (query-half qh in 0..1).
Each core computes all heads for its 1024 query rows against the full
2048 keys of its batch, entirely locally (no collectives).

On-device layout is fully "transposed" (k or d on partitions, q on the
free axis) so that the P@V matmul needs no transposes:
  S^T[k,q] = K_h^T.T @ q_h^T   (fp32r, contraction 64, 2 heads share the
                                128 partitions of the K/Q tiles)
  mask:    S^T += I_fp8 @ (-30*mask^T)_fp8  (PSUM accumulate)
  P^T     = exp(0.125 * S^T)   (ACT, PSUM->SBUF, f32r out)
  [out^T_h | denom_h] = [V_h | 1].T @ P^T  (PSUM accumulate over k tiles)
  coverage^T += P^T * bcast(1/(8*denom))   (DVE)
  out^T    = (8*Wc^T).T @ (out^T_h * bcast(1/(8*denom)))
Host side only does layout prep (transposes) and the inverse on outputs.
"""
import sys
sys.path.insert(0, '/opt/trn_rl_repo')
sys.path.insert(0, '/opt/pypackages')

import numpy as np

B, LQ, LK, D, H = 4, 2048, 2048, 512, 8
DH = D // H          # 64
NCORES = 8
QR = LQ // 2         # q rows per core = 1024
NKT = LK // 128      # 16 k tiles
NQB = 4              # q blocks per core
QB = QR // NQB       # 256
NIT = D // 128       # 4 partition tiles of the model dim

_nc_cache = None


def _build_nc():
    import concourse.bass as bass
    import concourse.bacc as bacc
    import concourse.tile as tile
    from concourse import mybir

    f32 = mybir.dt.float32
    f32r = mybir.dt.float32r
    fp8 = mybir.dt.float8e4
    u8 = mybir.dt.uint8
    AF = mybir.ActivationFunctionType
    ALU = mybir.AluOpType

    nc = bacc.Bacc("TRN2", target_bir_lowering=False, debug=False)

    qT_d = nc.dram_tensor("qT", [D, QR], f32, kind="ExternalInput").ap()
    kT_d = nc.dram_tensor("kT", [D, LK], f32, kind="ExternalInput").ap()
    v_d = nc.dram_tensor("v", [LK, D], f32, kind="ExternalInput").ap()
    mT_d = nc.dram_tensor("mT", [LK, QR], u8, kind="ExternalInput").ap()
    wqT_d = nc.dram_tensor("wqT", [D, D], f32, kind="ExternalInput").ap()
    wcT8_d = nc.dram_tensor("wcT8", [D, D], f32, kind="ExternalInput").ap()
    eye_d = nc.dram_tensor("eyeu8", [128, 128], u8, kind="ExternalInput").ap()

    outT_d = nc.dram_tensor("outT", [D, QR], f32, kind="ExternalOutput").ap()
    covT_d = nc.dram_tensor("covT", [LK, QR], f32, kind="ExternalOutput").ap()

    def ap3(t, part_rows, dims, extra_off=0):
        """Build a custom AP on tile t: partition slice + free [step,count] list."""
        base = t[:]
        p0 = base.ap[0]
        return bass.AP(tensor=base.tensor, offset=base.offset + extra_off,
                       ap=[[p0[0], part_rows]] + dims)

    with tile.TileContext(nc) as tc:
        with (
            tc.tile_pool(name="const", bufs=1) as const,
            tc.tile_pool(name="pers", bufs=1) as pers,
            tc.tile_pool(name="pp", bufs=6) as ppool,
            tc.tile_pool(name="small", bufs=2) as small,
            tc.tile_pool(name="psS", bufs=2, space="PSUM") as psS,
            tc.tile_pool(name="psPV", bufs=2, space="PSUM") as psPV,
            tc.tile_pool(name="psX", bufs=2, space="PSUM") as psX,
        ):
            # ---------------- constants ----------------
            ones_f = const.tile([128, 128], f32)
            nc.vector.memset(ones_f[:], 1.0)
            ones_r = const.tile([128, 128], f32r)
            nc.vector.tensor_copy(ones_r[:], ones_f[:])

            # ---------------- setup loads ----------------
            with tc.tile_pool(name="setupA", bufs=2) as setupA:
                # q raw + rounded (needed first: q-projection gates head 0)
                qraw = []
                for it in range(NIT):
                    t0 = setupA.tile([128, QR], f32, tag="qraw")
                    nc.sync.dma_start(t0[:], qT_d[it * 128:(it + 1) * 128, :])
                    t1 = setupA.tile([128, QR], f32r, tag="qrnd", bufs=NIT)
                    nc.vector.tensor_copy(t1[:], t0[:])
                    qraw.append(t1)
                # Wq
                wqr = []
                for it in range(NIT):
                    t0 = setupA.tile([128, D], f32, tag="wraw")
                    nc.sync.dma_start(t0[:], wqT_d[it * 128:(it + 1) * 128, :])
                    t1 = setupA.tile([128, D], f32r, tag="wrnd", bufs=NIT)
                    nc.vector.tensor_copy(t1[:], t0[:])
                    wqr.append(t1)
                # eye -> fp8
                eye8 = const.tile([128, 128], fp8)
                nc.sync.dma_start(eye8[:].bitcast(u8), eye_d)
                nc.vector.tensor_copy(eye8[:], eye8[:].bitcast(u8))

                # q projection: qTr[ot][:, q] = sum_it wqT[it][:, ot*128:...] ^T @ qraw[it]
                qTr = [pers.tile([128, QR], f32r, tag=f"qTr{ot}", name=f"qTr{ot}")
                       for ot in range(NIT)]
                for ot in range(NIT):
                    for qh2 in range(NQB):
                        pj = psX.tile([128, QB], f32, tag="x")
                        for it in range(NIT):
                            nc.tensor.matmul(
                                pj[:], wqr[it][:, ot * 128:(ot + 1) * 128],
                                qraw[it][:, qh2 * QB:(qh2 + 1) * QB],
                                start=(it == 0), stop=(it == NIT - 1))
                        nc.scalar.activation(
                            qTr[ot][:, qh2 * QB:(qh2 + 1) * QB], pj[:], AF.Copy)

            with tc.tile_pool(name="setupB", bufs=2) as setup:
                # K^T load + round
                kTr = []
                for it in range(NIT):
                    t0 = setup.tile([128, LK], f32, tag="kraw")
                    nc.sync.dma_start(t0[:], kT_d[it * 128:(it + 1) * 128, :])
                    t1 = pers.tile([128, LK], f32r, tag=f"kTr{it}", name=f"kTr{it}")
                    nc.vector.tensor_copy(t1[:], t0[:])
                    kTr.append(t1)

                # mask -> -30 * mask as fp8 (fp8 consumers: no f32r rounding rule)
                maskb = []
                for kt in range(NKT):
                    t1 = pers.tile([128, QR], fp8, tag=f"maskb{kt}", name=f"maskb{kt}")
                    nc.sync.dma_start(t1[:].bitcast(u8), mT_d[kt * 128:(kt + 1) * 128, :])
                    nc.vector.tensor_scalar_mul(t1[:], t1[:].bitcast(u8), -240.0)
                    maskb.append(t1)

                # V' = [V_h | 1] per head, interleaved: [128, h*65 + (0..64)]
                vpr = []
                for kt in range(NKT):
                    t0 = setup.tile([128, D], f32, tag="vraw")
                    nc.sync.dma_start(t0[:], v_d[kt * 128:(kt + 1) * 128, :])
                    t1 = pers.tile([128, H * 65], f32r, tag=f"vpr{kt}", name=f"vpr{kt}")
                    nc.vector.tensor_copy(
                        ap3(t1, 128, [[65, H], [1, DH]]),
                        ap3(t0, 128, [[DH, H], [1, DH]]))
                    nc.vector.tensor_copy(
                        ap3(t1, 128, [[65, H], [1, 1]], extra_off=DH),
                        ap3(ones_f, 128, [[0, H], [1, 1]]))
                    vpr.append(t1)

                # Wc^T * 8
                wcr = []
                for it in range(NIT):
                    t0 = setup.tile([128, D], f32, tag="wcraw")
                    nc.sync.dma_start(t0[:], wcT8_d[it * 128:(it + 1) * 128, :])
                    t1 = pers.tile([128, D], f32r, tag=f"wcr{it}", name=f"wcr{it}")
                    nc.vector.tensor_copy(t1[:], t0[:])
                    wcr.append(t1)

            # ---------------- persistent main-loop tensors ----------------
            OT = [pers.tile([128, QR], f32r, tag=f"OT{it}", name=f"OT{it}") for it in range(NIT)]
            C = pers.tile([128, NKT, QB], f32, tag="C")
            # row scratch at base partition 64 (reciprocal etc. of the denom row)
            rs = pers.tile([128, QB], f32, tag="rs")
            rs_r = pers.tile([128, QB], f32r, tag="rsr")

            NCH = 4          # pp chunks per head
            KCH = NKT // NCH  # 4 k-tiles per chunk

            for qb in range(NQB):
                qsl = slice(qb * QB, (qb + 1) * QB)
                for h in range(H):
                    it = h // 2
                    r0 = (h % 2) * 64
                    chunks = [ppool.tile([128, KCH, QB], f32r, tag="pp", name="ppc")
                              for _ in range(NCH)]
                    pv = psPV.tile([128, QB], f32, tag="pv")
                    for ci in range(NCH):          # S groups of KCH k-tiles
                        sg = psS.tile([128, KCH, QB], f32, tag="sg")
                        for j in range(KCH):
                            kt = ci * KCH + j
                            nc.tensor.matmul(
                                sg[:, j, :],
                                kTr[it][r0:r0 + 64, kt * 128:(kt + 1) * 128],
                                qTr[it][r0:r0 + 64, qsl],
                                start=True, stop=False)
                            nc.tensor.matmul(
                                sg[:, j, :], eye8[:], maskb[kt][:, qsl],
                                start=False, stop=True, skip_group_check=True)
                        nc.scalar.activation(
                            chunks[ci][:], sg[:], AF.Exp, scale=0.125)
                        for j in range(KCH):
                            kt = ci * KCH + j
                            nc.tensor.matmul(
                                pv[0:65, :],
                                vpr[kt][:, h * 65:(h + 1) * 65],
                                chunks[ci][:, j, :],
                                start=(kt == 0), stop=(kt == NKT - 1),
                                skip_group_check=True)
                    # denom -> reciprocal -> r/8 (all on partition row 64)
                    nc.vector.reciprocal(rs[64:65, :], pv[64:65, :])
                    nc.vector.tensor_scalar_mul(
                        rs_r[64:65, :], rs[64:65, :], 0.125)
                    rb8 = psX.tile([128, QB], f32, tag="x")
                    nc.tensor.matmul(
                        rb8[:], ones_r[64:65, 0:128], rs_r[64:65, 0:QB],
                        start=True, stop=True, skip_group_check=True)
                    rb8s = small.tile([128, QB], f32, tag="rb8s")
                    nc.scalar.activation(rb8s[:], rb8[:], AF.Copy)
                    # out^T_h = pv[0:64] * rb8s  (off by 1/8; fixed by 8*Wc)
                    otmp = small.tile([64, QB], f32r, tag="otmp")
                    nc.vector.tensor_tensor(
                        otmp[:], pv[0:64, :], rb8s[0:64, :], ALU.mult)
                    nc.sync.dma_start(OT[it][r0:r0 + 64, qsl], otmp[:])
                    # coverage: P' = P * rb8 (in place), C += P'
                    for ci in range(NCH):
                        rb8b = ap3(rb8s, 128, [[0, KCH], [1, QB]])
                        nc.vector.tensor_tensor(
                            chunks[ci][:], chunks[ci][:].bitcast(f32), rb8b,
                            ALU.mult)
                        csl = C[:, ci * KCH:(ci + 1) * KCH, :]
                        if h == 0:
                            nc.vector.tensor_copy(
                                csl, chunks[ci][:].bitcast(f32))
                        else:
                            nc.vector.tensor_tensor(
                                csl, csl, chunks[ci][:].bitcast(f32), ALU.add)

                # final projection for this q block
                for ot in range(NIT):
                    fp = psX.tile([128, QB], f32, tag="x")
                    for it2 in range(NIT):
                        nc.tensor.matmul(
                            fp[:], wcr[it2][:, ot * 128:(ot + 1) * 128],
                            OT[it2][:, qsl],
                            start=(it2 == 0), stop=(it2 == NIT - 1),
                            skip_group_check=True)
                    fout = small.tile([128, QB], f32, tag="fout")
                    nc.scalar.activation(fout[:], fp[:], AF.Copy)
                    nc.sync.dma_start(
                        outT_d[ot * 128:(ot + 1) * 128, qsl], fout[:])

                # coverage out for this q block: C [128, 16, 512] -> covT[k, q]
                cov_out = bass.AP(
                    tensor=covT_d.tensor, offset=qb * QB,
                    ap=[[QR, 128], [128 * QR, NKT], [1, QB]])
                nc.gpsimd.dma_start(cov_out, C[:])

    nc.compile()
    return nc


def kernel(query, key, value, mask, Wq, Wc):
    global _nc_cache
    from concourse import bass_utils

    query = np.asarray(query, dtype=np.float32)
    key = np.asarray(key, dtype=np.float32)
    value = np.asarray(value, dtype=np.float32)
    mask = np.asarray(mask)
    Wq = np.asarray(Wq, dtype=np.float32)
    Wc = np.asarray(Wc, dtype=np.float32)

    if _nc_cache is None:
        _nc_cache = _build_nc()
    nc = _nc_cache

    wqT = np.ascontiguousarray(Wq.T)
    wcT8 = np.ascontiguousarray(Wc.T) * 8.0
    eyeu8 = np.eye(128, dtype=np.uint8)

    in_maps = []
    for c in range(NCORES):
        b, qh = c // 2, c % 2
        qsl = slice(qh * QR, (qh + 1) * QR)
        in_maps.append(dict(
            qT=np.ascontiguousarray(query[b, qsl, :].T),
            kT=np.ascontiguousarray(key[b].T),
            v=np.ascontiguousarray(value[b]),
            mT=np.ascontiguousarray(mask[b, qsl, :].T).astype(np.uint8),
            wqT=wqT, wcT8=wcT8, eyeu8=eyeu8,
        ))

    res = bass_utils.run_bass_kernel_spmd(
        nc, in_maps, core_ids=list(range(NCORES)))

    out = np.empty((B, LQ, D), np.float32)
    cov = np.empty((B, LQ, LK), np.float32)
    for c in range(NCORES):
        b, qh = c // 2, c % 2
        qsl = slice(qh * QR, (qh + 1) * QR)
        out[b, qsl, :] = res.results[c]["outT"].T
        cov[b, qsl, :] = res.results[c]["covT"].T
    return out, cov


# revision 9
# speedup vs baseline: 3052.8285x; 3052.8285x over previous
"""Bass/Trainium2 kernel for masked multi-head attention with coverage.

Problem shapes (hardcoded): B=4, LQ=LK=2048, D=512, H=8, DH=64.
Sharding: 8 cores = (batch b in 0..3) x (query-half qh in 0..1).
Each core computes all heads for its 1024 query rows against the full
2048 keys of its batch, entirely locally (no collectives).

On-device layout is fully "transposed" (k or d on partitions, q on the
free axis) so that the P@V matmul needs no transposes:
  S^T[k,q] = K_h^T.T @ q_h^T   (fp32r, contraction 64, 2 heads share the
                                128 partitions of the K/Q tiles)
  mask:    S^T += I_fp8 @ (-30*mask^T)_fp8  (PSUM accumulate)
  P^T     = exp(0.125 * S^T)   (ACT, PSUM->SBUF, f32r out)
  [out^T_h | denom_h] = [V_h | 1].T @ P^T  (PSUM accumulate over k tiles)
  coverage^T += P^T * bcast(1/(8*denom))   (DVE)
  out^T    = (8*Wc^T).T @ (out^T_h * bcast(1/(8*denom)))
Host side only does layout prep (transposes) and the inverse on outputs.
"""
import sys
sys.path.insert(0, '/opt/trn_rl_repo')
sys.path.insert(0, '/opt/pypackages')

import numpy as np

B, LQ, LK, D, H = 4, 2048, 2048, 512, 8
DH = D // H          # 64
NCORES = 8
QR = LQ // 2         # q rows per core = 1024
NKT = LK // 128      # 16 k tiles
NQB = 4              # q blocks per core
QB = QR // NQB       # 256
NIT = D // 128       # 4 partition tiles of the model dim

_nc_cache = None


def _build_nc(cfg=None):
    cfg = cfg or {}
    PP_BUFS = cfg.get('pp_bufs', 6)
    SG_BUFS = cfg.get('sg_bufs', 2)
    NO_COV = cfg.get('no_cov', False)
    NO_MASK = cfg.get('no_mask', False)
    import concourse.bass as bass
    import concourse.bacc as bacc
    import concourse.tile as tile
    from concourse import mybir

    f32 = mybir.dt.float32
    f32r = mybir.dt.float32r
    fp8 = mybir.dt.float8e4
    u8 = mybir.dt.uint8
    AF = mybir.ActivationFunctionType
    ALU = mybir.AluOpType

    nc = bacc.Bacc("TRN2", target_bir_lowering=False, debug=False)

    qT_d = nc.dram_tensor("qT", [D, QR], f32, kind="ExternalInput").ap()
    kT_d = nc.dram_tensor("kT", [D, LK], f32, kind="ExternalInput").ap()
    v_d = nc.dram_tensor("v", [LK, D], f32, kind="ExternalInput").ap()
    mT_d = nc.dram_tensor("mT", [LK, QR], u8, kind="ExternalInput").ap()
    wqT_d = nc.dram_tensor("wqT", [D, D], f32, kind="ExternalInput").ap()
    wcT8_d = nc.dram_tensor("wcT8", [D, D], f32, kind="ExternalInput").ap()
    eye_d = nc.dram_tensor("eyeu8", [128, 128], u8, kind="ExternalInput").ap()

    outT_d = nc.dram_tensor("outT", [D, QR], f32, kind="ExternalOutput").ap()
    covT_d = nc.dram_tensor("covT", [LK, QR], f32, kind="ExternalOutput").ap()

    def ap3(t, part_rows, dims, extra_off=0):
        """Build a custom AP on tile t: partition slice + free [step,count] list."""
        base = t[:]
        p0 = base.ap[0]
        return bass.AP(tensor=base.tensor, offset=base.offset + extra_off,
                       ap=[[p0[0], part_rows]] + dims)

    with tile.TileContext(nc) as tc:
        with (
            tc.tile_pool(name="const", bufs=1) as const,
            tc.tile_pool(name="pers", bufs=1) as pers,
            tc.tile_pool(name="pp", bufs=PP_BUFS) as ppool,
            tc.tile_pool(name="small", bufs=2) as small,
            tc.tile_pool(name="psS", bufs=SG_BUFS, space="PSUM") as psS,
            tc.tile_pool(name="psPV", bufs=2, space="PSUM") as psPV,
            tc.tile_pool(name="psX", bufs=2, space="PSUM") as psX,
        ):
            # ---------------- constants ----------------
            ones_f = const.tile([128, 128], f32)
            nc.vector.memset(ones_f[:], 1.0)
            ones_r = const.tile([128, 128], f32r)
            nc.vector.tensor_copy(ones_r[:], ones_f[:])

            # ---------------- setup loads ----------------
            with tc.tile_pool(name="setupA", bufs=2) as setupA:
                # q raw + rounded (needed first: q-projection gates head 0)
                qraw = []
                for it in range(NIT):
                    t0 = setupA.tile([128, QR], f32, tag="qraw")
                    nc.sync.dma_start(t0[:], qT_d[it * 128:(it + 1) * 128, :])
                    t1 = setupA.tile([128, QR], f32r, tag="qrnd", bufs=NIT)
                    nc.vector.tensor_copy(t1[:], t0[:])
                    qraw.append(t1)
                # Wq
                wqr = []
                for it in range(NIT):
                    t0 = setupA.tile([128, D], f32, tag="wraw")
                    nc.sync.dma_start(t0[:], wqT_d[it * 128:(it + 1) * 128, :])
                    t1 = setupA.tile([128, D], f32r, tag="wrnd", bufs=NIT)
                    nc.vector.tensor_copy(t1[:], t0[:])
                    wqr.append(t1)
                # eye -> fp8
                eye8 = const.tile([128, 128], fp8)
                nc.sync.dma_start(eye8[:].bitcast(u8), eye_d)
                nc.vector.tensor_copy(eye8[:], eye8[:].bitcast(u8))

                # q projection: qTr[ot][:, q] = sum_it wqT[it][:, ot*128:...] ^T @ qraw[it]
                qTr = [pers.tile([128, QR], f32r, tag=f"qTr{ot}", name=f"qTr{ot}")
                       for ot in range(NIT)]
                for ot in range(NIT):
                    for qh2 in range(NQB):
                        pj = psX.tile([128, QB], f32, tag="x")
                        for it in range(NIT):
                            nc.tensor.matmul(
                                pj[:], wqr[it][:, ot * 128:(ot + 1) * 128],
                                qraw[it][:, qh2 * QB:(qh2 + 1) * QB],
                                start=(it == 0), stop=(it == NIT - 1))
                        nc.scalar.activation(
                            qTr[ot][:, qh2 * QB:(qh2 + 1) * QB], pj[:], AF.Copy)

            with tc.tile_pool(name="setupB", bufs=2) as setup:
                # K^T load + round
                kTr = []
                for it in range(NIT):
                    t0 = setup.tile([128, LK], f32, tag="kraw")
                    nc.sync.dma_start(t0[:], kT_d[it * 128:(it + 1) * 128, :])
                    t1 = pers.tile([128, LK], f32r, tag=f"kTr{it}", name=f"kTr{it}")
                    nc.vector.tensor_copy(t1[:], t0[:])
                    kTr.append(t1)

                # mask -> -30 * mask as fp8 (fp8 consumers: no f32r rounding rule)
                maskb = []
                for kt in range(NKT):
                    t1 = pers.tile([128, QR], fp8, tag=f"maskb{kt}", name=f"maskb{kt}")
                    nc.sync.dma_start(t1[:].bitcast(u8), mT_d[kt * 128:(kt + 1) * 128, :])
                    nc.vector.tensor_scalar_mul(t1[:], t1[:].bitcast(u8), -240.0)
                    maskb.append(t1)

                # V' = [V_h | 1] per head, interleaved: [128, h*65 + (0..64)]
                vpr = []
                for kt in range(NKT):
                    t0 = setup.tile([128, D], f32, tag="vraw")
                    nc.sync.dma_start(t0[:], v_d[kt * 128:(kt + 1) * 128, :])
                    t1 = pers.tile([128, H * 65], f32r, tag=f"vpr{kt}", name=f"vpr{kt}")
                    nc.vector.tensor_copy(
                        ap3(t1, 128, [[65, H], [1, DH]]),
                        ap3(t0, 128, [[DH, H], [1, DH]]))
                    nc.vector.tensor_copy(
                        ap3(t1, 128, [[65, H], [1, 1]], extra_off=DH),
                        ap3(ones_f, 128, [[0, H], [1, 1]]))
                    vpr.append(t1)

                # Wc^T * 8
                wcr = []
                for it in range(NIT):
                    t0 = setup.tile([128, D], f32, tag="wcraw")
                    nc.sync.dma_start(t0[:], wcT8_d[it * 128:(it + 1) * 128, :])
                    t1 = pers.tile([128, D], f32r, tag=f"wcr{it}", name=f"wcr{it}")
                    nc.vector.tensor_copy(t1[:], t0[:])
                    wcr.append(t1)

            # ---------------- persistent main-loop tensors ----------------
            OT = [pers.tile([128, QR], f32r, tag=f"OT{it}", name=f"OT{it}") for it in range(NIT)]
            C = pers.tile([128, NKT, QB], f32, tag="C")
            # row scratch at base partition 64 (reciprocal etc. of the denom row)
            rs = pers.tile([128, QB], f32, tag="rs")
            rs_r = pers.tile([128, QB], f32r, tag="rsr")

            NCH = 4          # pp chunks per head
            KCH = NKT // NCH  # 4 k-tiles per chunk

            for qb in range(NQB):
                qsl = slice(qb * QB, (qb + 1) * QB)
                for h in range(H):
                    it = h // 2
                    r0 = (h % 2) * 64
                    chunks = [ppool.tile([128, KCH, QB], f32r, tag="pp", name="ppc")
                              for _ in range(NCH)]
                    pv = psPV.tile([128, QB], f32, tag="pv")
                    for ci in range(NCH):          # S groups of KCH k-tiles
                        sg = psS.tile([128, KCH, QB], f32, tag="sg")
                        for j in range(KCH):
                            kt = ci * KCH + j
                            nc.tensor.matmul(
                                sg[:, j, :],
                                kTr[it][r0:r0 + 64, kt * 128:(kt + 1) * 128],
                                qTr[it][r0:r0 + 64, qsl],
                                start=True, stop=NO_MASK)
                            if NO_MASK:
                                pass
                            else:
                                nc.tensor.matmul(
                                    sg[:, j, :], eye8[:], maskb[kt][:, qsl],
                                    start=False, stop=True, skip_group_check=True)
                        nc.scalar.activation(
                            chunks[ci][:], sg[:], AF.Exp, scale=0.125)
                        for j in range(KCH):
                            kt = ci * KCH + j
                            nc.tensor.matmul(
                                pv[0:65, :],
                                vpr[kt][:, h * 65:(h + 1) * 65],
                                chunks[ci][:, j, :],
                                start=(kt == 0), stop=(kt == NKT - 1),
                                skip_group_check=True)
                    # denom -> reciprocal -> r/8 (all on partition row 64)
                    nc.vector.reciprocal(rs[64:65, :], pv[64:65, :])
                    nc.vector.tensor_scalar_mul(
                        rs_r[64:65, :], rs[64:65, :], 0.125)
                    rb8 = psX.tile([128, QB], f32, tag="x")
                    nc.tensor.matmul(
                        rb8[:], ones_r[64:65, 0:128], rs_r[64:65, 0:QB],
                        start=True, stop=True, skip_group_check=True)
                    rb8s = small.tile([128, QB], f32, tag="rb8s")
                    nc.scalar.activation(rb8s[:], rb8[:], AF.Copy)
                    # out^T_h = pv[0:64] * rb8s  (off by 1/8; fixed by 8*Wc)
                    otmp = small.tile([64, QB], f32r, tag="otmp")
                    nc.vector.tensor_tensor(
                        otmp[:], pv[0:64, :], rb8s[0:64, :], ALU.mult)
                    nc.sync.dma_start(OT[it][r0:r0 + 64, qsl], otmp[:])
                    # coverage: P' = P * rb8, C += P' (h=0 writes C directly)
                    for ci in range(NCH):
                        if NO_COV:
                            break
                        rb8b = ap3(rb8s, 128, [[0, KCH], [1, QB]])
                        csl = C[:, ci * KCH:(ci + 1) * KCH, :]
                        if h == 0:
                            nc.vector.tensor_tensor(
                                csl, chunks[ci][:].bitcast(f32), rb8b,
                                ALU.mult)
                        else:
                            nc.vector.tensor_tensor(
                                chunks[ci][:], chunks[ci][:].bitcast(f32), rb8b,
                                ALU.mult)
                            nc.vector.tensor_tensor(
                                csl, csl, chunks[ci][:].bitcast(f32), ALU.add)

                # final projection for this q block
                for ot in range(NIT):
                    fp = psX.tile([128, QB], f32, tag="x")
                    for it2 in range(NIT):
                        nc.tensor.matmul(
                            fp[:], wcr[it2][:, ot * 128:(ot + 1) * 128],
                            OT[it2][:, qsl],
                            start=(it2 == 0), stop=(it2 == NIT - 1),
                            skip_group_check=True)
                    fout = small.tile([128, QB], f32, tag="fout")
                    nc.scalar.activation(fout[:], fp[:], AF.Copy)
                    nc.sync.dma_start(
                        outT_d[ot * 128:(ot + 1) * 128, qsl], fout[:])

                # coverage out for this q block: C [128, 16, 512] -> covT[k, q]
                if not NO_COV:
                    cov_out = bass.AP(
                        tensor=covT_d.tensor, offset=qb * QB,
                        ap=[[QR, 128], [128 * QR, NKT], [1, QB]])
                    nc.gpsimd.dma_start(cov_out, C[:])

    nc.compile()
    return nc


def kernel(query, key, value, mask, Wq, Wc):
    global _nc_cache
    from concourse import bass_utils

    query = np.asarray(query, dtype=np.float32)
    key = np.asarray(key, dtype=np.float32)
    value = np.asarray(value, dtype=np.float32)
    mask = np.asarray(mask)
    Wq = np.asarray(Wq, dtype=np.float32)
    Wc = np.asarray(Wc, dtype=np.float32)

    if _nc_cache is None:
        _nc_cache = _build_nc()
    nc = _nc_cache

    wqT = np.ascontiguousarray(Wq.T)
    wcT8 = np.ascontiguousarray(Wc.T) * 8.0
    eyeu8 = np.eye(128, dtype=np.uint8)

    in_maps = []
    for c in range(NCORES):
        b, qh = c // 2, c % 2
        qsl = slice(qh * QR, (qh + 1) * QR)
        in_maps.append(dict(
            qT=np.ascontiguousarray(query[b, qsl, :].T),
            kT=np.ascontiguousarray(key[b].T),
            v=np.ascontiguousarray(value[b]),
            mT=np.ascontiguousarray(mask[b, qsl, :].T).astype(np.uint8),
            wqT=wqT, wcT8=wcT8, eyeu8=eyeu8,
        ))

    res = bass_utils.run_bass_kernel_spmd(
        nc, in_maps, core_ids=list(range(NCORES)))

    out = np.empty((B, LQ, D), np.float32)
    cov = np.empty((B, LQ, LK), np.float32)
    for c in range(NCORES):
        b, qh = c // 2, c % 2
        qsl = slice(qh * QR, (qh + 1) * QR)
        out[b, qsl, :] = res.results[c]["outT"].T
        cov[b, qsl, :] = res.results[c]["covT"].T
    return out, cov


# revision 10
# speedup vs baseline: 73928.0277x; 24.2162x over previous
"""Bass/Trainium2 kernel for masked multi-head attention with coverage.

Problem shapes (hardcoded): B=4, LQ=LK=2048, D=512, H=8, DH=64.
Sharding: 8 cores = (batch b in 0..3) x (query-half qh in 0..1).
Each core computes all heads for its 1024 query rows against the full
2048 keys of its batch, entirely locally (no collectives).

On-device layout is fully "transposed" (k or d on partitions, q on the
free axis) so that the P@V matmul needs no transposes:
  S^T[k,q] = K_h^T.T @ q_h^T   (fp32r, contraction 64, 2 heads share the
                                128 partitions of the K/Q tiles)
  mask:    S^T += I_fp8 @ (-30*mask^T)_fp8  (PSUM accumulate)
  P^T     = exp(0.125 * S^T)   (ACT, PSUM->SBUF, f32r out)
  [out^T_h | denom_h] = [V_h | 1].T @ P^T  (PSUM accumulate over k tiles)
  coverage^T += P^T * bcast(1/(8*denom))   (DVE)
  out^T    = (8*Wc^T).T @ (out^T_h * bcast(1/(8*denom)))
Host side only does layout prep (transposes) and the inverse on outputs.
"""
import sys
sys.path.insert(0, '/opt/trn_rl_repo')
sys.path.insert(0, '/opt/pypackages')

import numpy as np

B, LQ, LK, D, H = 4, 2048, 2048, 512, 8
DH = D // H          # 64
NCORES = 8
QR = LQ // 2         # q rows per core = 1024
NKT = LK // 128      # 16 k tiles
NQB = 4              # q blocks per core
QB = QR // NQB       # 256
NIT = D // 128       # 4 partition tiles of the model dim

_nc_cache = None


def _build_nc(cfg=None):
    cfg = cfg or {}
    PP_BUFS = cfg.get('pp_bufs', 6)
    SG_BUFS = cfg.get('sg_bufs', 2)
    NO_COV = cfg.get('no_cov', False)
    NO_MASK = cfg.get('no_mask', False)
    REPS = cfg.get('reps', 1)
    import concourse.bass as bass
    import concourse.bacc as bacc
    import concourse.tile as tile
    from concourse import mybir

    f32 = mybir.dt.float32
    f32r = mybir.dt.float32r
    fp8 = mybir.dt.float8e4
    u8 = mybir.dt.uint8
    AF = mybir.ActivationFunctionType
    ALU = mybir.AluOpType

    nc = bacc.Bacc("TRN2", target_bir_lowering=False, debug=False)

    qT_d = nc.dram_tensor("qT", [D, QR], f32, kind="ExternalInput").ap()
    kT_d = nc.dram_tensor("kT", [D, LK], f32, kind="ExternalInput").ap()
    v_d = nc.dram_tensor("v", [LK, D], f32, kind="ExternalInput").ap()
    mT_d = nc.dram_tensor("mT", [LK, QR], u8, kind="ExternalInput").ap()
    wqT_d = nc.dram_tensor("wqT", [D, D], f32, kind="ExternalInput").ap()
    wcT8_d = nc.dram_tensor("wcT8", [D, D], f32, kind="ExternalInput").ap()
    eye_d = nc.dram_tensor("eyeu8", [128, 128], u8, kind="ExternalInput").ap()

    outT_d = nc.dram_tensor("outT", [D, QR], f32, kind="ExternalOutput").ap()
    covT_d = nc.dram_tensor("covT", [LK, QR], f32, kind="ExternalOutput").ap()

    def ap3(t, part_rows, dims, extra_off=0):
        """Build a custom AP on tile t: partition slice + free [step,count] list."""
        base = t[:]
        p0 = base.ap[0]
        return bass.AP(tensor=base.tensor, offset=base.offset + extra_off,
                       ap=[[p0[0], part_rows]] + dims)

    with tile.TileContext(nc) as tc:
      for _rep in range(REPS):
        with (
            tc.tile_pool(name="const", bufs=1) as const,
            tc.tile_pool(name="pers", bufs=1) as pers,
            tc.tile_pool(name="pp", bufs=PP_BUFS) as ppool,
            tc.tile_pool(name="small", bufs=2) as small,
            tc.tile_pool(name="psS", bufs=SG_BUFS, space="PSUM") as psS,
            tc.tile_pool(name="psPV", bufs=2, space="PSUM") as psPV,
            tc.tile_pool(name="psX", bufs=2, space="PSUM") as psX,
        ):
            # ---------------- constants ----------------
            ones_f = const.tile([128, 128], f32)
            nc.vector.memset(ones_f[:], 1.0)
            ones_r = const.tile([128, 128], f32r)
            nc.vector.tensor_copy(ones_r[:], ones_f[:])

            # ---------------- setup loads ----------------
            with tc.tile_pool(name="setupA", bufs=2) as setupA:
                # q raw + rounded (needed first: q-projection gates head 0)
                qraw = []
                for it in range(NIT):
                    t0 = setupA.tile([128, QR], f32, tag="qraw")
                    nc.sync.dma_start(t0[:], qT_d[it * 128:(it + 1) * 128, :])
                    t1 = setupA.tile([128, QR], f32r, tag="qrnd", bufs=NIT)
                    nc.vector.tensor_copy(t1[:], t0[:])
                    qraw.append(t1)
                # Wq
                wqr = []
                for it in range(NIT):
                    t0 = setupA.tile([128, D], f32, tag="wraw")
                    nc.sync.dma_start(t0[:], wqT_d[it * 128:(it + 1) * 128, :])
                    t1 = setupA.tile([128, D], f32r, tag="wrnd", bufs=NIT)
                    nc.vector.tensor_copy(t1[:], t0[:])
                    wqr.append(t1)
                # eye -> fp8
                eye8 = const.tile([128, 128], fp8)
                nc.sync.dma_start(eye8[:].bitcast(u8), eye_d)
                nc.vector.tensor_copy(eye8[:], eye8[:].bitcast(u8))

                # q projection: qTr[ot][:, q] = sum_it wqT[it][:, ot*128:...] ^T @ qraw[it]
                qTr = [pers.tile([128, QR], f32r, tag=f"qTr{ot}", name=f"qTr{ot}")
                       for ot in range(NIT)]
                for ot in range(NIT):
                    for qh2 in range(NQB):
                        pj = psX.tile([128, QB], f32, tag="x")
                        for it in range(NIT):
                            nc.tensor.matmul(
                                pj[:], wqr[it][:, ot * 128:(ot + 1) * 128],
                                qraw[it][:, qh2 * QB:(qh2 + 1) * QB],
                                start=(it == 0), stop=(it == NIT - 1))
                        nc.scalar.activation(
                            qTr[ot][:, qh2 * QB:(qh2 + 1) * QB], pj[:], AF.Copy)

            with tc.tile_pool(name="setupB", bufs=2) as setup:
                # K^T load + round
                kTr = []
                for it in range(NIT):
                    t0 = setup.tile([128, LK], f32, tag="kraw")
                    nc.sync.dma_start(t0[:], kT_d[it * 128:(it + 1) * 128, :])
                    t1 = pers.tile([128, LK], f32r, tag=f"kTr{it}", name=f"kTr{it}")
                    nc.vector.tensor_copy(t1[:], t0[:])
                    kTr.append(t1)

                # mask -> -30 * mask as fp8 (fp8 consumers: no f32r rounding rule)
                maskb = []
                for kt in range(NKT):
                    t1 = pers.tile([128, QR], fp8, tag=f"maskb{kt}", name=f"maskb{kt}")
                    nc.sync.dma_start(t1[:].bitcast(u8), mT_d[kt * 128:(kt + 1) * 128, :])
                    nc.vector.tensor_scalar_mul(t1[:], t1[:].bitcast(u8), -240.0)
                    maskb.append(t1)

                # V' = [V_h | 1] per head, interleaved: [128, h*65 + (0..64)]
                vpr = []
                for kt in range(NKT):
                    t0 = setup.tile([128, D], f32, tag="vraw")
                    nc.sync.dma_start(t0[:], v_d[kt * 128:(kt + 1) * 128, :])
                    t1 = pers.tile([128, H * 65], f32r, tag=f"vpr{kt}", name=f"vpr{kt}")
                    nc.vector.tensor_copy(
                        ap3(t1, 128, [[65, H], [1, DH]]),
                        ap3(t0, 128, [[DH, H], [1, DH]]))
                    nc.vector.tensor_copy(
                        ap3(t1, 128, [[65, H], [1, 1]], extra_off=DH),
                        ap3(ones_f, 128, [[0, H], [1, 1]]))
                    vpr.append(t1)

                # Wc^T * 8
                wcr = []
                for it in range(NIT):
                    t0 = setup.tile([128, D], f32, tag="wcraw")
                    nc.sync.dma_start(t0[:], wcT8_d[it * 128:(it + 1) * 128, :])
                    t1 = pers.tile([128, D], f32r, tag=f"wcr{it}", name=f"wcr{it}")
                    nc.vector.tensor_copy(t1[:], t0[:])
                    wcr.append(t1)

            # ---------------- persistent main-loop tensors ----------------
            OT = [pers.tile([128, QR], f32r, tag=f"OT{it}", name=f"OT{it}") for it in range(NIT)]
            C = pers.tile([128, NKT, QB], f32, tag="C")
            # row scratch at base partition 64 (reciprocal etc. of the denom row)
            rs = pers.tile([128, QB], f32, tag="rs")
            rs_r = pers.tile([128, QB], f32r, tag="rsr")

            NCH = 4          # pp chunks per head
            KCH = NKT // NCH  # 4 k-tiles per chunk

            for qb in range(NQB):
                qsl = slice(qb * QB, (qb + 1) * QB)
                for h in range(H):
                    it = h // 2
                    r0 = (h % 2) * 64
                    chunks = [ppool.tile([128, KCH, QB], f32r, tag="pp", name="ppc")
                              for _ in range(NCH)]
                    pv = psPV.tile([128, QB], f32, tag="pv")
                    for ci in range(NCH):          # S groups of KCH k-tiles
                        sg = psS.tile([128, KCH, QB], f32, tag="sg")
                        for j in range(KCH):
                            kt = ci * KCH + j
                            nc.tensor.matmul(
                                sg[:, j, :],
                                kTr[it][r0:r0 + 64, kt * 128:(kt + 1) * 128],
                                qTr[it][r0:r0 + 64, qsl],
                                start=True, stop=NO_MASK)
                            if NO_MASK:
                                pass
                            else:
                                nc.tensor.matmul(
                                    sg[:, j, :], eye8[:], maskb[kt][:, qsl],
                                    start=False, stop=True, skip_group_check=True)
                        nc.scalar.activation(
                            chunks[ci][:], sg[:], AF.Exp, scale=0.125)
                        for j in range(KCH):
                            kt = ci * KCH + j
                            nc.tensor.matmul(
                                pv[0:65, :],
                                vpr[kt][:, h * 65:(h + 1) * 65],
                                chunks[ci][:, j, :],
                                start=(kt == 0), stop=(kt == NKT - 1),
                                skip_group_check=True)
                    # denom -> reciprocal -> r/8 (all on partition row 64)
                    nc.vector.reciprocal(rs[64:65, :], pv[64:65, :])
                    nc.vector.tensor_scalar_mul(
                        rs_r[64:65, :], rs[64:65, :], 0.125)
                    rb8 = psX.tile([128, QB], f32, tag="x")
                    nc.tensor.matmul(
                        rb8[:], ones_r[64:65, 0:128], rs_r[64:65, 0:QB],
                        start=True, stop=True, skip_group_check=True)
                    rb8s = small.tile([128, QB], f32, tag="rb8s")
                    nc.scalar.activation(rb8s[:], rb8[:], AF.Copy)
                    # out^T_h = pv[0:64] * rb8s  (off by 1/8; fixed by 8*Wc)
                    otmp = small.tile([64, QB], f32r, tag="otmp")
                    nc.vector.tensor_tensor(
                        otmp[:], pv[0:64, :], rb8s[0:64, :], ALU.mult)
                    nc.sync.dma_start(OT[it][r0:r0 + 64, qsl], otmp[:])
                    # coverage: P' = P * rb8, C += P' (h=0 writes C directly)
                    for ci in range(NCH):
                        if NO_COV:
                            break
                        rb8b = ap3(rb8s, 128, [[0, KCH], [1, QB]])
                        csl = C[:, ci * KCH:(ci + 1) * KCH, :]
                        if h == 0:
                            nc.vector.tensor_tensor(
                                csl, chunks[ci][:].bitcast(f32), rb8b,
                                ALU.mult)
                        else:
                            nc.vector.tensor_tensor(
                                chunks[ci][:], chunks[ci][:].bitcast(f32), rb8b,
                                ALU.mult)
                            nc.vector.tensor_tensor(
                                csl, csl, chunks[ci][:].bitcast(f32), ALU.add)

                # final projection for this q block
                for ot in range(NIT):
                    fp = psX.tile([128, QB], f32, tag="x")
                    for it2 in range(NIT):
                        nc.tensor.matmul(
                            fp[:], wcr[it2][:, ot * 128:(ot + 1) * 128],
                            OT[it2][:, qsl],
                            start=(it2 == 0), stop=(it2 == NIT - 1),
                            skip_group_check=True)
                    fout = small.tile([128, QB], f32, tag="fout")
                    nc.scalar.activation(fout[:], fp[:], AF.Copy)
                    nc.sync.dma_start(
                        outT_d[ot * 128:(ot + 1) * 128, qsl], fout[:])

                # coverage out for this q block: C [128, 16, 512] -> covT[k, q]
                if not NO_COV:
                    cov_out = bass.AP(
                        tensor=covT_d.tensor, offset=qb * QB,
                        ap=[[QR, 128], [128 * QR, NKT], [1, QB]])
                    nc.gpsimd.dma_start(cov_out, C[:])

    nc.compile()
    return nc


def kernel(query, key, value, mask, Wq, Wc):
    global _nc_cache
    from concourse import bass_utils

    query = np.asarray(query, dtype=np.float32)
    key = np.asarray(key, dtype=np.float32)
    value = np.asarray(value, dtype=np.float32)
    mask = np.asarray(mask)
    Wq = np.asarray(Wq, dtype=np.float32)
    Wc = np.asarray(Wc, dtype=np.float32)

    if _nc_cache is None:
        _nc_cache = _build_nc()
    nc = _nc_cache

    wqT = np.ascontiguousarray(Wq.T)
    wcT8 = np.ascontiguousarray(Wc.T) * 8.0
    eyeu8 = np.eye(128, dtype=np.uint8)

    in_maps = []
    for c in range(NCORES):
        b, qh = c // 2, c % 2
        qsl = slice(qh * QR, (qh + 1) * QR)
        in_maps.append(dict(
            qT=np.ascontiguousarray(query[b, qsl, :].T),
            kT=np.ascontiguousarray(key[b].T),
            v=np.ascontiguousarray(value[b]),
            mT=np.ascontiguousarray(mask[b, qsl, :].T).astype(np.uint8),
            wqT=wqT, wcT8=wcT8, eyeu8=eyeu8,
        ))

    res = bass_utils.run_bass_kernel_spmd(
        nc, in_maps, core_ids=list(range(NCORES)))

    out = np.empty((B, LQ, D), np.float32)
    cov = np.empty((B, LQ, LK), np.float32)
    for c in range(NCORES):
        b, qh = c // 2, c % 2
        qsl = slice(qh * QR, (qh + 1) * QR)
        out[b, qsl, :] = res.results[c]["outT"].T
        cov[b, qsl, :] = res.results[c]["covT"].T
    return out, cov
